# revision 18
# baseline (speedup 1.0000x reference)
"""Causal self-attention (GQA + RMS-norm + RoPE) Trainium2 Bass kernel.

Sharding: 8 cores = 4 batches x 2 head-groups (tensor-parallel over heads).
Core c = 2*b + t handles batch b with Q heads [8t, 8t+8) and KV heads
[2t, 2t+2). Each core computes a partial output projection (its heads'
rows of W_out); the host sums the two partials per batch.

All matmul operands are bf16 (fp32 PSUM accumulation); elementwise /
normalization math stays fp32. Everything (qT, kT, v, ytn) is SBUF
resident -- no DRAM scratch round-trip.

Pipeline per core:
  P1: qkv = x @ W_shard (transposed-x input), RMS+RoPE on q/k in natural
      layout, PE-transpose q/k to [d, tok] into resident SBUF.
  P2: per 512-token query window, per head: scoresT = kT_tile.T @ qT_win,
      +tri-mask on diagonal tiles, exp (ACT, scale=hd^-0.5), then
      yT += v_tile.T @ expT; per-window sums for all 8 heads accumulate
      into one [8,512] PSUM bank via one-hot stationary columns; a single
      Ln/Exp pair per window gives 1/sums, broadcast via one-hot-row
      matmuls, applied on DVE.
  P3: out = sum_h yT_norm_h.T @ W_out_h rows -> partial [S, D] (bf16).

Activation-table note: all ACT functions used (Square/Ln/Exp/Copy) live
in the 'natural_log_exp_and_others' table; we reorder the table list fed
to the act-table-load pass so that table is preferred (and remap the
emitted ids back to act_info.json order), avoiding per-activation table
reloads.
"""
import sys, os
sys.path.insert(0, '/opt/trn_rl_repo')
import numpy as np
import ml_dtypes

import concourse.bacc as bacc_mod
from concourse import bass, bacc, mybir, tile

f32 = mybir.dt.float32
bf16 = mybir.dt.bfloat16
BF = ml_dtypes.bfloat16

B, S, D = 4, 2048, 2048
H, HKV, HD = 16, 4, 128
HLOC = H // 2          # 8 q heads per core
KVLOC = HKV // 2       # 2 kv heads per core
SCALE = float(HD) ** -0.5
RMS_EPS = float(np.finfo(np.float32).eps)
ROPE_BASE = 10000.0

NTC = S // 128         # 16 token tiles
NDT = D // 128         # 16 contraction tiles
NWIN = S // 512        # 4 query windows


# ---- activation-table preference patch: prefer the table holding
# square+ln+exp+copy so the kernel needs a single ACT_TABLE_LOAD. ----
_PREF_TABLE = 'natural_log_exp_and_others'
_orig_insert_atl = bacc_mod._bass_rust.insert_act_table_loads


def _patched_insert_atl(bobj, tables):
    names = [t[0] for t in tables]
    if _PREF_TABLE in names:
        pi = names.index(_PREF_TABLE)
        order = [pi] + [i for i in range(len(tables)) if i != pi]
        _orig_insert_atl(bobj, [tables[i] for i in order])
        for blk in bobj.main_func.blocks:
            for inst in blk.instructions:
                if type(inst).__name__ == 'InstLoadActFuncSet':
                    inst.act_func_set_id = order[inst.act_func_set_id]
    else:
        _orig_insert_atl(bobj, tables)


bacc_mod._bass_rust.insert_act_table_loads = _patched_insert_atl


def _rope_tables():
    inv_freq = (1.0 / (ROPE_BASE ** (np.arange(0, HD, 2, dtype=np.float32) / HD))).astype(np.float32)
    freqs = np.arange(S, dtype=np.float32)[:, None] * inv_freq[None, :]
    cos = np.cos(freqs).astype(np.float32)
    sin = np.sin(freqs).astype(np.float32)
    cos2 = np.concatenate([cos, cos], axis=1)        # [S, 128]
    sin2 = np.concatenate([sin, -sin], axis=1)       # [S, 128]
    return cos2, sin2


def _tri_masks():
    # triangular mask for the 128-col diagonal block of a diag score tile:
    # local col f, row p: masked iff p > f
    m = np.zeros((128, 128), dtype=np.float32)
    p = np.arange(128)[:, None]
    f = np.arange(128)[None, :]
    m[p > f] = -1e30
    return m


def _emit_rms_rope(nc, scr, psum_ap, nheads, cos1, sin1, nat_tile, eps_ap):
    """psum_ap: [128, nheads*128] qkv psum slice; writes RMS+RoPE result into
    nat_tile (SBUF, bf16). cos1/sin1: [128, 1, 128] APs (cos dup, [sin,-sin]).

    rot(q) = q*cos2 + swap_halves(q)*sin2;  out = rot(q) * rsqrt(mean(q^2)+eps)
    rsqrt computed as exp(-0.5*ln(ss/128+eps)) on ACT.
    """
    w = nheads * 128
    sq = scr.tile([128, w], bf16, tag="sq")
    nc.scalar.activation(sq[:], psum_ap, mybir.ActivationFunctionType.Square)
    ss = scr.tile([128, nheads, 1], f32, tag="ss")
    nc.vector.tensor_reduce(
        ss[:], sq[:].rearrange("p (h f) -> p h f", h=nheads),
        axis=mybir.AxisListType.X, op=mybir.AluOpType.add)
    lg = scr.tile([128, nheads, 1], f32, tag="lg")
    nc.scalar.activation(lg[:], ss[:], mybir.ActivationFunctionType.Ln,
                         scale=1.0 / HD, bias=eps_ap)
    rinv = scr.tile([128, nheads, 1, 1], f32, tag="rinv")
    nc.scalar.activation(rinv[:], lg[:], mybir.ActivationFunctionType.Exp,
                         scale=-0.5)

    shp = [128, nheads, 2, 64]
    p4 = psum_ap.rearrange("p (h x f) -> p h x f", h=nheads, x=2)
    p4s = p4[:, :, ::-1, :]
    cb = cos1.rearrange("p t (x f) -> p t x f", x=2).to_broadcast(shp)
    sb_ = sin1.rearrange("p t (x f) -> p t x f", x=2).to_broadcast(shp)
    rb = rinv[:].to_broadcast(shp)
    t1 = scr.tile(shp, bf16, tag="t1")
    t2 = scr.tile(shp, bf16, tag="t2")
    nc.vector.tensor_mul(t1[:], p4, cb)
    nc.vector.tensor_mul(t2[:], p4s, sb_)
    nc.vector.tensor_add(t1[:], t1[:], t2[:])
    nc.vector.tensor_mul(nat_tile[:].rearrange("p (h x f) -> p h x f", h=nheads, x=2),
                         t1[:], rb)


def build_program():
    cos_np, sin_np = _rope_tables()
    masks_np = _tri_masks()
    # one-hot column matrices for batched sums: oh8[p, h, c] = (c == h)
    oh_col = np.zeros((128, HLOC, HLOC), dtype=np.float32)
    for h in range(HLOC):
        oh_col[:, h, h] = 1.0
    # one-hot row matrices for broadcast: ohr[p, h, c] = (p == h)
    oh_row = np.zeros((HLOC, HLOC, 128), dtype=np.float32)
    for h in range(HLOC):
        oh_row[h, h, :] = 1.0

    nc = bacc.Bacc(trn_type="TRN2")

    xt_d = nc.dram_tensor("xt", [D, S], bf16, kind="ExternalInput")
    wq_d = nc.dram_tensor("wq", [D, HLOC * HD], bf16, kind="ExternalInput")
    wkv_d = nc.dram_tensor("wkv", [D, 2 * KVLOC * HD], bf16, kind="ExternalInput")
    wo_d = nc.dram_tensor("wo", [HLOC * HD, D], bf16, kind="ExternalInput")
    out_d = nc.dram_tensor("out", [S, D], bf16, kind="ExternalOutput")

    cos_d = nc.inline_tensor(cos_np.astype(BF), "cos_t")
    sin_d = nc.inline_tensor(sin_np.astype(BF), "sin_t")
    ident_d = nc.inline_tensor(np.eye(128, dtype=np.float32).astype(BF), "ident")
    masks_d = nc.inline_tensor(masks_np.astype(BF), "tri_masks")
    ohc_d = nc.inline_tensor(oh_col.astype(BF), "oh_col")
    ohr_d = nc.inline_tensor(oh_row.astype(BF), "oh_row")

    with tile.TileContext(nc) as tc:
        with tc.tile_pool(name="cst", bufs=1) as cst:
            cos_sb = cst.tile([128, NTC, 128], bf16, tag="cos")
            sin_sb = cst.tile([128, NTC, 128], bf16, tag="sin")
            ident = cst.tile([128, 128], bf16, tag="ident")
            masks = cst.tile([128, 128], bf16, tag="masks")
            ohc = cst.tile([128, HLOC, HLOC], bf16, tag="ohc")
            ohr = cst.tile([HLOC, HLOC, 128], bf16, tag="ohr")
            eps_sb = cst.tile([128, 1], f32, tag="eps")

            nc.gpsimd.dma_start(out=ident[:], in_=ident_d[:])
            nc.gpsimd.dma_start(out=masks[:], in_=masks_d[:])
            nc.gpsimd.dma_start(out=ohc[:], in_=ohc_d[:])
            nc.gpsimd.dma_start(out=ohr[:], in_=ohr_d[:])
            nc.gpsimd.memset(eps_sb[:], RMS_EPS)

            # resident tensors
            qt_sb = cst.tile([128, HLOC, S], bf16, tag="qt")
            kt_sb = cst.tile([128, KVLOC, S], bf16, tag="kt")
            v_sb = cst.tile([128, NTC, KVLOC * HD], bf16, tag="v")
            ytn = cst.tile([128, HLOC, S], bf16, tag="ytn")
            wq_sb = cst.tile([128, NDT, HLOC * HD], bf16, tag="wq")
            wkv_sb = cst.tile([128, NDT, 512], bf16, tag="wkv")
            wo_sb = cst.tile([128, HLOC, D], bf16, tag="wo")

            # weight preloads: per-slice, spread across idle queues so the
            # first matmuls can start as soon as their slices land
            wkv_r = wkv_d[:].rearrange("(t p) c -> p t c", p=128)
            wq_r = wq_d[:].rearrange("(t p) c -> p t c", p=128)
            wo_r = wo_d[:].rearrange("(h p) c -> p h c", p=128)
            for dt in range(NDT):
                nc.gpsimd.dma_start(out=wkv_sb[:, dt, :], in_=wkv_r[:, dt, :])
            for dt in range(0, NDT, 2):
                nc.scalar.dma_start(out=wq_sb[:, dt, :], in_=wq_r[:, dt, :])
            for h in range(HLOC):
                nc.gpsimd.dma_start(out=wo_sb[:, h, :], in_=wo_r[:, h, :])

            # ---- fused per-window pipeline ----
            # for each 512-token window w: P1 (qkv+rms+rope+transpose for its
            # 4 token tiles) -> P2 attention over windows's queries -> batched
            # softmax normalization -> P3 (output projection) for window w-1,
            # interleaved into P2 of the NEXT window as PE gap filler.
            with tc.tile_pool(name="xs", bufs=2) as xs, \
                 tc.tile_pool(name="nat", bufs=2) as nat, \
                 tc.tile_pool(name="ex", bufs=12) as ex, \
                 tc.tile_pool(name="sm", bufs=1) as sm, \
                 tc.tile_pool(name="eu", bufs=4) as eu, \
                 tc.tile_pool(name="ob", bufs=2) as ob, \
                 tc.tile_pool(name="acc", bufs=2, space="PSUM") as acc, \
                 tc.tile_pool(name="psc", bufs=3, space="PSUM") as psc, \
                 tc.tile_pool(name="py", bufs=2, space="PSUM") as py, \
                 tc.tile_pool(name="pn", bufs=1, space="PSUM") as pn:

                # prefetch x for the first two token tiles ahead of the
                # constant/weight loads so the first matmuls start immediately
                xt_pre = {}
                for tcid in (0, 1):
                    xt_sb = xs.tile([128, NDT, 128], bf16, tag="xt")
                    nc.sync.dma_start(
                        out=xt_sb[:],
                        in_=xt_d[:, tcid * 128:(tcid + 1) * 128]
                            .rearrange("(t p) s -> p t s", p=128))
                    xt_pre[tcid] = xt_sb
                nc.sync.dma_start(out=cos_sb[:], in_=cos_d[:].rearrange("(t p) f -> p t f", p=128))
                nc.sync.dma_start(out=sin_sb[:], in_=sin_d[:].rearrange("(t p) f -> p t f", p=128))
                for dt in range(1, NDT, 2):
                    nc.sync.dma_start(out=wq_sb[:, dt, :], in_=wq_r[:, dt, :])

                def emit_p1_group(ps, nheads, cos1, sin1, heads):
                    # RMS+RoPE on psum group, then PE-transpose each head tile
                    # into its resident [d, tok] slot. heads: list of
                    # (dst_tile, dst_head, col0, tcid)
                    qn = nat.tile([128, nheads * 128], bf16, tag="qn")
                    _emit_rms_rope(nc, nat, ps, nheads, cos1, sin1, qn, eps_sb[:])
                    for idx, (dst, dh, c0, tcid) in enumerate(heads):
                        tp = psc.tile([128, 128], bf16, tag="sc")
                        nc.tensor.transpose(tp[:], qn[:, c0:c0 + 128], ident[:])
                        if idx % 2 == 0:
                            nc.vector.tensor_copy(dst[:, dh, tcid * 128:(tcid + 1) * 128], tp[:])
                        else:
                            nc.scalar.activation(dst[:, dh, tcid * 128:(tcid + 1) * 128], tp[:],
                                                 mybir.ActivationFunctionType.Copy)

                def emit_p1_tc(tcid):
                    if tcid in xt_pre:
                        xt_sb = xt_pre.pop(tcid)
                    else:
                        xt_sb = xs.tile([128, NDT, 128], bf16, tag="xt")
                        nc.sync.dma_start(
                            out=xt_sb[:],
                            in_=xt_d[:, tcid * 128:(tcid + 1) * 128]
                                .rearrange("(t p) s -> p t s", p=128))
                    cos1 = cos_sb[:, tcid:tcid + 1, :]
                    sin1 = sin_sb[:, tcid:tcid + 1, :]
                    # group order: window 0 runs kv first (wkv is only 2MB
                    # on its own DMA queue) so the PE has work during the wq
                    # load ramp; later windows run q groups first.
                    def emit_kv():
                        ps_kv = acc.tile([128, 512], f32, tag="acc")
                        for dt in range(NDT):
                            nc.tensor.matmul(ps_kv[:], xt_sb[:, dt, :], wkv_sb[:, dt, :],
                                             start=dt == 0, stop=dt == NDT - 1)
                        emit_p1_group(ps_kv[:, 0:256], 2, cos1, sin1,
                                      [(kt_sb, kh, kh * 128, tcid) for kh in range(KVLOC)])
                        nc.vector.tensor_copy(v_sb[:, tcid, :], ps_kv[:, 256:512])

                    def emit_q(gi):
                        ps_q = acc.tile([128, 512], f32, tag="acc")
                        for dt in range(NDT):
                            nc.tensor.matmul(ps_q[:], xt_sb[:, dt, :],
                                             wq_sb[:, dt, gi * 512:(gi + 1) * 512],
                                             start=dt == 0, stop=dt == NDT - 1)
                        emit_p1_group(ps_q[:], 4, cos1, sin1,
                                      [(qt_sb, gi * 4 + hh, hh * 128, tcid) for hh in range(4)])

                    if tcid < 4:
                        emit_kv(); emit_q(0); emit_q(1)
                    else:
                        emit_q(0); emit_q(1); emit_kv()

                def emit_p3_tile(og, tcid):
                    ps_o = acc.tile([128, 512], f32, tag="acc")
                    for h in range(HLOC):
                        nc.tensor.matmul(
                            ps_o[:],
                            ytn[:, h, tcid * 128:(tcid + 1) * 128],
                            wo_sb[:, h, og * 512:(og + 1) * 512],
                            start=(h == 0), stop=(h == HLOC - 1))
                    ot = ob.tile([128, 512], bf16, tag="ot")
                    nc.vector.tensor_copy(ot[:], ps_o[:])
                    nc.gpsimd.dma_start(
                        out=out_d[tcid * 128:(tcid + 1) * 128, og * 512:(og + 1) * 512],
                        in_=ot[:])

                for w in range(NWIN):
                    for tcid in range(4 * w, 4 * w + 4):
                        emit_p1_tc(tcid)

                    # ---- P2 window w (+ P3 of window w-1 as gap filler) ----
                    njt = 4 * w + 4
                    ps_sums = pn.tile([HLOC, 512], f32, tag="sums")
                    for hp in range(HLOC // 2):
                        h0, h1 = 2 * hp, 2 * hp + 1
                        kvh = h0 // 4
                        ps_y0 = py.tile([128, 512], f32, tag="y")
                        ps_y1 = py.tile([128, 512], f32, tag="y")
                        # software pipeline: scores/exp for tile j are
                        # emitted BEFORE the y/sums matmuls of tile j-1, so
                        # the in-order PE queue never waits on exp latency.
                        saved = {}
                        pend_sums = {h0: [], h1: []}
                        first_sum = {h0: True, h1: True}
                        for j in range(njt + 1):
                            if j < njt:
                                vi = j - 4 * w
                                c0 = 128 * vi if vi >= 0 else 0
                                kt_j = kt_sb[:, kvh, j * 128:(j + 1) * 128]
                                cur = {}
                                for hq in (h0, h1):
                                    ps_sc = psc.tile([128, 512], f32, tag="sc")
                                    nc.tensor.matmul(
                                        ps_sc[:, c0:512], kt_j,
                                        qt_sb[:, hq, w * 512 + c0:(w + 1) * 512])
                                    if vi >= 0:
                                        nc.vector.tensor_add(ps_sc[:, c0:c0 + 128],
                                                             ps_sc[:, c0:c0 + 128],
                                                             masks[:])
                                    et = ex.tile([128, 512], bf16, tag="et")
                                    nc.scalar.activation(et[:, c0:512], ps_sc[:, c0:512],
                                                         mybir.ActivationFunctionType.Exp,
                                                         scale=SCALE)
                                    cur[hq] = (et, c0)
                                saved[j] = cur
                            if j == 0:
                                continue
                            jp = j - 1
                            v_jp = v_sb[:, jp, kvh * 128:(kvh + 1) * 128]
                            st, sp = jp == 0, jp == njt - 1
                            for hq, ps_y in ((h0, ps_y0), (h1, ps_y1)):
                                et, c0e = saved[jp][hq]
                                nc.tensor.matmul(
                                    ps_y[:, c0e:512], v_jp,
                                    et[:, c0e:512], start=st, stop=sp,
                                    skip_group_check=True)
                                # sums: group et tiles on DVE before the
                                # ones-matmul -- 4-wide for non-diagonal tiles,
                                # pairs for diagonal ones -- cutting the PE
                                # streaming for sums accumulation
                                grp = pend_sums[hq]
                                grp.append((et, c0e))
                                ndiag = jp < 4 * w
                                if ndiag and len(grp) == 4:
                                    a1 = eu.tile([128, 512], bf16, tag="au")
                                    nc.vector.tensor_add(a1[:], grp[0][0][:], grp[1][0][:])
                                    a2 = eu.tile([128, 512], bf16, tag="au")
                                    nc.vector.tensor_add(a2[:], grp[2][0][:], grp[3][0][:])
                                    au = eu.tile([128, 512], bf16, tag="au")
                                    nc.vector.tensor_add(au[:], a1[:], a2[:])
                                    su, cs = au, 0
                                    pend_sums[hq] = []
                                elif (not ndiag) and len(grp) == 2:
                                    (e0, ca), (e1, cb) = grp
                                    au = eu.tile([128, 512], bf16, tag="au")
                                    if cb > ca:
                                        nc.vector.tensor_copy(au[:, ca:cb], e0[:, ca:cb])
                                    nc.vector.tensor_add(au[:, cb:512],
                                                         e0[:, cb:512], e1[:, cb:512])
                                    su, cs = au, ca
                                    pend_sums[hq] = []
                                else:
                                    su = None
                                if su is not None:
                                    nc.tensor.matmul(
                                        ps_sums[:, cs:512], ohc[:, hq, :], su[:, cs:512],
                                        start=(hq == 0 and first_sum[hq]),
                                        stop=(hq == HLOC - 1 and sp),
                                        skip_group_check=True)
                                    first_sum[hq] = False
                            del saved[jp]
                        nc.scalar.activation(ytn[:, h0, w * 512:(w + 1) * 512], ps_y0[:],
                                             mybir.ActivationFunctionType.Copy)
                        nc.scalar.activation(ytn[:, h1, w * 512:(w + 1) * 512], ps_y1[:],
                                             mybir.ActivationFunctionType.Copy)
                        # P3 gap filler: one og-stripe of the previous window
                        if w > 0:
                            for tcl in range(4):
                                emit_p3_tile(hp, 4 * (w - 1) + tcl)

                    # batched 1/sums for all 8 heads of this window
                    lgs = sm.tile([HLOC, 512], f32, tag="lgs")
                    nc.scalar.activation(lgs[:], ps_sums[:],
                                         mybir.ActivationFunctionType.Ln)
                    rec = sm.tile([HLOC, 512], bf16, tag="rec")
                    nc.scalar.activation(rec[:], lgs[:],
                                         mybir.ActivationFunctionType.Exp,
                                         scale=-1.0)
                    for hq in range(HLOC):
                        bcp = psc.tile([128, 512], f32, tag="sc")
                        nc.tensor.matmul(bcp[:], ohr[:, hq, :], rec[:])
                        nc.vector.tensor_mul(
                            ytn[:, hq, w * 512:(w + 1) * 512],
                            ytn[:, hq, w * 512:(w + 1) * 512], bcp[:])

                # final P3 stripe: window 3
                for og in range(4):
                    for tcl in range(4):
                        emit_p3_tile(og, 12 + tcl)

    nc.compile()
    return nc


_PROGRAM = None


def _get_program():
    global _PROGRAM
    if _PROGRAM is None:
        _PROGRAM = build_program()
    return _PROGRAM


def make_in_maps(x, W_qkv, W_out):
    in_maps = []
    for c in range(8):
        b, t = c // 2, c % 2
        xt = np.ascontiguousarray(x[b].T).astype(BF)
        wq = np.ascontiguousarray(W_qkv[:, t * 1024:(t + 1) * 1024]).astype(BF)
        wk = W_qkv[:, D + t * 256: D + (t + 1) * 256]
        wv = W_qkv[:, D + 512 + t * 256: D + 512 + (t + 1) * 256]
        wkv = np.ascontiguousarray(np.concatenate([wk, wv], axis=1)).astype(BF)
        wo = np.ascontiguousarray(W_out[t * 1024:(t + 1) * 1024, :]).astype(BF)
        in_maps.append({"xt": xt, "wq": wq, "wkv": wkv, "wo": wo})
    return in_maps


def kernel(x, W_qkv, W_out):
    from concourse.bass_utils import run_bass_kernel_spmd
    nc = _get_program()
    in_maps = make_in_maps(np.asarray(x, dtype=np.float32),
                           np.asarray(W_qkv, dtype=np.float32),
                           np.asarray(W_out, dtype=np.float32))
    res = run_bass_kernel_spmd(nc, in_maps, list(range(8)), trace=False)
    out = np.empty((B, S, D), dtype=np.float32)
    for b in range(B):
        out[b] = (res.results[2 * b]["out"].astype(np.float32)
                  + res.results[2 * b + 1]["out"].astype(np.float32))
    return out


# revision 20
# speedup vs baseline: 1.0094x; 1.0094x over previous
"""Causal self-attention (GQA + RMS-norm + RoPE) Trainium2 Bass kernel.

Sharding: 8 cores = 4 batches x 2 head-groups (tensor-parallel over heads).
Core c = 2*b + t handles batch b with Q heads [8t, 8t+8) and KV heads
[2t, 2t+2). Each core computes a partial output projection (its heads'
rows of W_out); the host sums the two partials per batch.

All matmul operands are bf16 (fp32 PSUM accumulation); elementwise /
normalization math stays fp32. Everything (qT, kT, v, ytn) is SBUF
resident -- no DRAM scratch round-trip.

Pipeline per core:
  P1: qkv = x @ W_shard (transposed-x input), RMS+RoPE on q/k in natural
      layout, PE-transpose q/k to [d, tok] into resident SBUF.
  P2: per 512-token query window, per head: scoresT = kT_tile.T @ qT_win,
      +tri-mask on diagonal tiles, exp (ACT, scale=hd^-0.5), then
      yT += v_tile.T @ expT; per-window sums for all 8 heads accumulate
      into one [8,512] PSUM bank via one-hot stationary columns; a single
      Ln/Exp pair per window gives 1/sums, broadcast via one-hot-row
      matmuls, applied on DVE.
  P3: out = sum_h yT_norm_h.T @ W_out_h rows -> partial [S, D] (bf16).

Activation-table note: all ACT functions used (Square/Ln/Exp/Copy) live
in the 'natural_log_exp_and_others' table; we reorder the table list fed
to the act-table-load pass so that table is preferred (and remap the
emitted ids back to act_info.json order), avoiding per-activation table
reloads.
"""
import sys, os
sys.path.insert(0, '/opt/trn_rl_repo')
import numpy as np
import ml_dtypes

import concourse.bacc as bacc_mod
from concourse import bass, bacc, mybir, tile

f32 = mybir.dt.float32
bf16 = mybir.dt.bfloat16
BF = ml_dtypes.bfloat16

B, S, D = 4, 2048, 2048
H, HKV, HD = 16, 4, 128
HLOC = H // 2          # 8 q heads per core
KVLOC = HKV // 2       # 2 kv heads per core
SCALE = float(HD) ** -0.5
RMS_EPS = float(np.finfo(np.float32).eps)
ROPE_BASE = 10000.0

NTC = S // 128         # 16 token tiles
NDT = D // 128         # 16 contraction tiles
NWIN = S // 512        # 4 query windows


# ---- activation-table preference patch: prefer the table holding
# square+ln+exp+copy so the kernel needs a single ACT_TABLE_LOAD. ----
_PREF_TABLE = 'natural_log_exp_and_others'
_orig_insert_atl = bacc_mod._bass_rust.insert_act_table_loads


def _patched_insert_atl(bobj, tables):
    names = [t[0] for t in tables]
    if _PREF_TABLE in names:
        pi = names.index(_PREF_TABLE)
        order = [pi] + [i for i in range(len(tables)) if i != pi]
        _orig_insert_atl(bobj, [tables[i] for i in order])
        for blk in bobj.main_func.blocks:
            for inst in blk.instructions:
                if type(inst).__name__ == 'InstLoadActFuncSet':
                    inst.act_func_set_id = order[inst.act_func_set_id]
    else:
        _orig_insert_atl(bobj, tables)


bacc_mod._bass_rust.insert_act_table_loads = _patched_insert_atl


def _rope_tables():
    inv_freq = (1.0 / (ROPE_BASE ** (np.arange(0, HD, 2, dtype=np.float32) / HD))).astype(np.float32)
    freqs = np.arange(S, dtype=np.float32)[:, None] * inv_freq[None, :]
    cos = np.cos(freqs).astype(np.float32)
    sin = np.sin(freqs).astype(np.float32)
    cos2 = np.concatenate([cos, cos], axis=1)        # [S, 128]
    sin2 = np.concatenate([sin, -sin], axis=1)       # [S, 128]
    return cos2, sin2


def _tri_masks():
    # triangular mask for the 128-col diagonal block of a diag score tile:
    # local col f, row p: masked iff p > f
    m = np.zeros((128, 128), dtype=np.float32)
    p = np.arange(128)[:, None]
    f = np.arange(128)[None, :]
    m[p > f] = -1e30
    return m


def _emit_rms_rope(nc, scr, psum_ap, nheads, cos1, sin1, nat_tile, eps_ap):
    """psum_ap: [128, nheads*128] qkv psum slice; writes RMS+RoPE result into
    nat_tile (SBUF, bf16). cos1/sin1: [128, 1, 128] APs (cos dup, [sin,-sin]).

    rot(q) = q*cos2 + swap_halves(q)*sin2;  out = rot(q) * rsqrt(mean(q^2)+eps)
    rsqrt computed as exp(-0.5*ln(ss/128+eps)) on ACT.
    """
    w = nheads * 128
    sq = scr.tile([128, w], bf16, tag="sq")
    nc.scalar.activation(sq[:], psum_ap, mybir.ActivationFunctionType.Square)
    ss = scr.tile([128, nheads, 1], f32, tag="ss")
    nc.vector.tensor_reduce(
        ss[:], sq[:].rearrange("p (h f) -> p h f", h=nheads),
        axis=mybir.AxisListType.X, op=mybir.AluOpType.add)
    lg = scr.tile([128, nheads, 1], f32, tag="lg")
    nc.scalar.activation(lg[:], ss[:], mybir.ActivationFunctionType.Ln,
                         scale=1.0 / HD, bias=eps_ap)
    rinv = scr.tile([128, nheads, 1, 1], f32, tag="rinv")
    nc.scalar.activation(rinv[:], lg[:], mybir.ActivationFunctionType.Exp,
                         scale=-0.5)

    shp = [128, nheads, 2, 64]
    p4 = psum_ap.rearrange("p (h x f) -> p h x f", h=nheads, x=2)
    p4s = p4[:, :, ::-1, :]
    cb = cos1.rearrange("p t (x f) -> p t x f", x=2).to_broadcast(shp)
    sb_ = sin1.rearrange("p t (x f) -> p t x f", x=2).to_broadcast(shp)
    rb = rinv[:].to_broadcast(shp)
    t1 = scr.tile(shp, bf16, tag="t1")
    t2 = scr.tile(shp, bf16, tag="t2")
    nc.vector.tensor_mul(t1[:], p4, cb)
    nc.vector.tensor_mul(t2[:], p4s, sb_)
    nc.vector.tensor_add(t1[:], t1[:], t2[:])
    nc.vector.tensor_mul(nat_tile[:].rearrange("p (h x f) -> p h x f", h=nheads, x=2),
                         t1[:], rb)


def build_program():
    cos_np, sin_np = _rope_tables()
    masks_np = _tri_masks()
    # one-hot column matrices for batched sums: oh8[p, h, c] = (c == h)
    oh_col = np.zeros((128, HLOC, HLOC), dtype=np.float32)
    for h in range(HLOC):
        oh_col[:, h, h] = 1.0
    # one-hot row matrices for broadcast: ohr[p, h, c] = (p == h)
    oh_row = np.zeros((HLOC, HLOC, 128), dtype=np.float32)
    for h in range(HLOC):
        oh_row[h, h, :] = 1.0

    nc = bacc.Bacc(trn_type="TRN2")

    xt_d = nc.dram_tensor("xt", [D, S], bf16, kind="ExternalInput")
    wq_d = nc.dram_tensor("wq", [D, HLOC * HD], bf16, kind="ExternalInput")
    wkv_d = nc.dram_tensor("wkv", [D, 2 * KVLOC * HD], bf16, kind="ExternalInput")
    wo_d = nc.dram_tensor("wo", [HLOC * HD, D], bf16, kind="ExternalInput")
    out_d = nc.dram_tensor("out", [S, D], bf16, kind="ExternalOutput")

    cos_d = nc.inline_tensor(cos_np.astype(BF), "cos_t")
    sin_d = nc.inline_tensor(sin_np.astype(BF), "sin_t")
    ident_d = nc.inline_tensor(np.eye(128, dtype=np.float32).astype(BF), "ident")
    masks_d = nc.inline_tensor(masks_np.astype(BF), "tri_masks")
    ohc_d = nc.inline_tensor(oh_col.astype(BF), "oh_col")
    ohr_d = nc.inline_tensor(oh_row.astype(BF), "oh_row")

    with tile.TileContext(nc) as tc:
        with tc.tile_pool(name="cst", bufs=1) as cst:
            cos_sb = cst.tile([128, NTC, 128], bf16, tag="cos")
            sin_sb = cst.tile([128, NTC, 128], bf16, tag="sin")
            ident = cst.tile([128, 128], bf16, tag="ident")
            masks = cst.tile([128, 128], bf16, tag="masks")
            ohc = cst.tile([128, HLOC, HLOC], bf16, tag="ohc")
            ohr = cst.tile([HLOC, HLOC, 128], bf16, tag="ohr")
            eps_sb = cst.tile([128, 1], f32, tag="eps")

            nc.gpsimd.dma_start(out=ident[:], in_=ident_d[:])
            nc.gpsimd.dma_start(out=masks[:], in_=masks_d[:])
            nc.gpsimd.dma_start(out=ohc[:], in_=ohc_d[:])
            nc.gpsimd.dma_start(out=ohr[:], in_=ohr_d[:])
            nc.gpsimd.memset(eps_sb[:], RMS_EPS)

            # resident tensors
            qt_sb = cst.tile([128, HLOC, S], bf16, tag="qt")
            kt_sb = cst.tile([128, KVLOC, S], bf16, tag="kt")
            v_sb = cst.tile([128, NTC, KVLOC * HD], bf16, tag="v")
            ytn = cst.tile([128, HLOC, S], bf16, tag="ytn")
            wq_sb = cst.tile([128, NDT, HLOC * HD], bf16, tag="wq")
            wkv_sb = cst.tile([128, NDT, 512], bf16, tag="wkv")
            wo_sb = cst.tile([128, HLOC, D], bf16, tag="wo")

            # weight preloads: per-slice, spread across idle queues so the
            # first matmuls can start as soon as their slices land
            wkv_r = wkv_d[:].rearrange("(t p) c -> p t c", p=128)
            wq_r = wq_d[:].rearrange("(t p) c -> p t c", p=128)
            wo_r = wo_d[:].rearrange("(h p) c -> p h c", p=128)
            for dt in range(NDT):
                nc.gpsimd.dma_start(out=wkv_sb[:, dt, :], in_=wkv_r[:, dt, :])
            for dt in range(0, NDT, 2):
                nc.scalar.dma_start(out=wq_sb[:, dt, :], in_=wq_r[:, dt, :])
            for h in range(HLOC):
                nc.gpsimd.dma_start(out=wo_sb[:, h, :], in_=wo_r[:, h, :])

            # ---- fused per-window pipeline ----
            # for each 512-token window w: P1 (qkv+rms+rope+transpose for its
            # 4 token tiles) -> P2 attention over windows's queries -> batched
            # softmax normalization -> P3 (output projection) for window w-1,
            # interleaved into P2 of the NEXT window as PE gap filler.
            with tc.tile_pool(name="xs", bufs=2) as xs, \
                 tc.tile_pool(name="nat", bufs=2) as nat, \
                 tc.tile_pool(name="ex", bufs=10) as ex, \
                 tc.tile_pool(name="sm", bufs=1) as sm, \
                 tc.tile_pool(name="eu", bufs=3) as eu, \
                 tc.tile_pool(name="ob", bufs=2) as ob, \
                 tc.tile_pool(name="acc", bufs=2, space="PSUM") as acc, \
                 tc.tile_pool(name="psc", bufs=3, space="PSUM") as psc, \
                 tc.tile_pool(name="py", bufs=2, space="PSUM") as py, \
                 tc.tile_pool(name="pn", bufs=1, space="PSUM") as pn:

                # HAM warmup: the first ~20us are DMA-bound (weights
                # streaming in); keep the PE array active on dummy identity
                # matmuls so the clock gate stays at full rate when real
                # work arrives.
                for _ in range(160):
                    wp = psc.tile([128, 128], f32, tag="sc")
                    nc.tensor.matmul(wp[:], ident[:], ident[:])

                # prefetch x for the first two token tiles ahead of the
                # constant/weight loads so the first matmuls start immediately
                xt_pre = {}
                for tcid in (0, 1):
                    xt_sb = xs.tile([128, NDT, 128], bf16, tag="xt")
                    nc.sync.dma_start(
                        out=xt_sb[:],
                        in_=xt_d[:, tcid * 128:(tcid + 1) * 128]
                            .rearrange("(t p) s -> p t s", p=128))
                    xt_pre[tcid] = xt_sb
                nc.sync.dma_start(out=cos_sb[:], in_=cos_d[:].rearrange("(t p) f -> p t f", p=128))
                nc.sync.dma_start(out=sin_sb[:], in_=sin_d[:].rearrange("(t p) f -> p t f", p=128))
                for dt in range(1, NDT, 2):
                    nc.sync.dma_start(out=wq_sb[:, dt, :], in_=wq_r[:, dt, :])

                def emit_p1_group(ps, nheads, cos1, sin1, heads):
                    # RMS+RoPE on psum group, then PE-transpose each head tile
                    # into its resident [d, tok] slot. heads: list of
                    # (dst_tile, dst_head, col0, tcid)
                    qn = nat.tile([128, nheads * 128], bf16, tag="qn")
                    _emit_rms_rope(nc, nat, ps, nheads, cos1, sin1, qn, eps_sb[:])
                    for idx, (dst, dh, c0, tcid) in enumerate(heads):
                        tp = psc.tile([128, 128], bf16, tag="sc")
                        nc.tensor.transpose(tp[:], qn[:, c0:c0 + 128], ident[:])
                        if idx % 2 == 0:
                            nc.vector.tensor_copy(dst[:, dh, tcid * 128:(tcid + 1) * 128], tp[:])
                        else:
                            nc.scalar.activation(dst[:, dh, tcid * 128:(tcid + 1) * 128], tp[:],
                                                 mybir.ActivationFunctionType.Copy)

                def emit_p1_tc(tcid):
                    if tcid in xt_pre:
                        xt_sb = xt_pre.pop(tcid)
                    else:
                        xt_sb = xs.tile([128, NDT, 128], bf16, tag="xt")
                        nc.sync.dma_start(
                            out=xt_sb[:],
                            in_=xt_d[:, tcid * 128:(tcid + 1) * 128]
                                .rearrange("(t p) s -> p t s", p=128))
                    cos1 = cos_sb[:, tcid:tcid + 1, :]
                    sin1 = sin_sb[:, tcid:tcid + 1, :]
                    # group order: window 0 runs kv first (wkv is only 2MB
                    # on its own DMA queue) so the PE has work during the wq
                    # load ramp; later windows run q groups first.
                    def emit_kv():
                        ps_kv = acc.tile([128, 512], f32, tag="acc")
                        for dt in range(NDT):
                            nc.tensor.matmul(ps_kv[:], xt_sb[:, dt, :], wkv_sb[:, dt, :],
                                             start=dt == 0, stop=dt == NDT - 1)
                        emit_p1_group(ps_kv[:, 0:256], 2, cos1, sin1,
                                      [(kt_sb, kh, kh * 128, tcid) for kh in range(KVLOC)])
                        nc.vector.tensor_copy(v_sb[:, tcid, :], ps_kv[:, 256:512])

                    def emit_q(gi):
                        ps_q = acc.tile([128, 512], f32, tag="acc")
                        for dt in range(NDT):
                            nc.tensor.matmul(ps_q[:], xt_sb[:, dt, :],
                                             wq_sb[:, dt, gi * 512:(gi + 1) * 512],
                                             start=dt == 0, stop=dt == NDT - 1)
                        emit_p1_group(ps_q[:], 4, cos1, sin1,
                                      [(qt_sb, gi * 4 + hh, hh * 128, tcid) for hh in range(4)])

                    if tcid < 4:
                        emit_kv(); emit_q(0); emit_q(1)
                    else:
                        emit_q(0); emit_q(1); emit_kv()

                def emit_p3_tile(og, tcid):
                    ps_o = acc.tile([128, 512], f32, tag="acc")
                    for h in range(HLOC):
                        nc.tensor.matmul(
                            ps_o[:],
                            ytn[:, h, tcid * 128:(tcid + 1) * 128],
                            wo_sb[:, h, og * 512:(og + 1) * 512],
                            start=(h == 0), stop=(h == HLOC - 1))
                    ot = ob.tile([128, 512], bf16, tag="ot")
                    nc.vector.tensor_copy(ot[:], ps_o[:])
                    nc.gpsimd.dma_start(
                        out=out_d[tcid * 128:(tcid + 1) * 128, og * 512:(og + 1) * 512],
                        in_=ot[:])

                for w in range(NWIN):
                    for tcid in range(4 * w, 4 * w + 4):
                        emit_p1_tc(tcid)

                    # ---- P2 window w (+ P3 of window w-1 as gap filler) ----
                    njt = 4 * w + 4
                    ps_sums = pn.tile([HLOC, 512], f32, tag="sums")
                    for hp in range(HLOC // 2):
                        h0, h1 = 2 * hp, 2 * hp + 1
                        kvh = h0 // 4
                        ps_y0 = py.tile([128, 512], f32, tag="y")
                        ps_y1 = py.tile([128, 512], f32, tag="y")
                        # software pipeline: scores/exp for tile j are
                        # emitted BEFORE the y/sums matmuls of tile j-1, so
                        # the in-order PE queue never waits on exp latency.
                        saved = {}
                        pend_sums = {h0: [], h1: []}
                        first_sum = {h0: True, h1: True}
                        for j in range(njt + 1):
                            if j < njt:
                                vi = j - 4 * w
                                c0 = 128 * vi if vi >= 0 else 0
                                kt_j = kt_sb[:, kvh, j * 128:(j + 1) * 128]
                                cur = {}
                                for hq in (h0, h1):
                                    ps_sc = psc.tile([128, 512], f32, tag="sc")
                                    nc.tensor.matmul(
                                        ps_sc[:, c0:512], kt_j,
                                        qt_sb[:, hq, w * 512 + c0:(w + 1) * 512])
                                    if vi >= 0:
                                        nc.vector.tensor_add(ps_sc[:, c0:c0 + 128],
                                                             ps_sc[:, c0:c0 + 128],
                                                             masks[:])
                                    et = ex.tile([128, 512], bf16, tag="et")
                                    nc.scalar.activation(et[:, c0:512], ps_sc[:, c0:512],
                                                         mybir.ActivationFunctionType.Exp,
                                                         scale=SCALE)
                                    cur[hq] = (et, c0)
                                saved[j] = cur
                            if j == 0:
                                continue
                            jp = j - 1
                            v_jp = v_sb[:, jp, kvh * 128:(kvh + 1) * 128]
                            st, sp = jp == 0, jp == njt - 1
                            for hq, ps_y in ((h0, ps_y0), (h1, ps_y1)):
                                et, c0e = saved[jp][hq]
                                nc.tensor.matmul(
                                    ps_y[:, c0e:512], v_jp,
                                    et[:, c0e:512], start=st, stop=sp,
                                    skip_group_check=True)
                                # sums: group et tiles on DVE before the
                                # ones-matmul -- 4-wide for non-diagonal tiles,
                                # pairs for diagonal ones -- cutting the PE
                                # streaming for sums accumulation
                                grp = pend_sums[hq]
                                grp.append((et, c0e))
                                ndiag = jp < 4 * w
                                if ndiag and len(grp) == 4:
                                    a1 = eu.tile([128, 512], bf16, tag="au")
                                    nc.vector.tensor_add(a1[:], grp[0][0][:], grp[1][0][:])
                                    a2 = eu.tile([128, 512], bf16, tag="au")
                                    nc.vector.tensor_add(a2[:], grp[2][0][:], grp[3][0][:])
                                    au = eu.tile([128, 512], bf16, tag="au")
                                    nc.vector.tensor_add(au[:], a1[:], a2[:])
                                    su, cs = au, 0
                                    pend_sums[hq] = []
                                elif (not ndiag) and len(grp) == 2:
                                    (e0, ca), (e1, cb) = grp
                                    au = eu.tile([128, 512], bf16, tag="au")
                                    if cb > ca:
                                        nc.vector.tensor_copy(au[:, ca:cb], e0[:, ca:cb])
                                    nc.vector.tensor_add(au[:, cb:512],
                                                         e0[:, cb:512], e1[:, cb:512])
                                    su, cs = au, ca
                                    pend_sums[hq] = []
                                else:
                                    su = None
                                if su is not None:
                                    nc.tensor.matmul(
                                        ps_sums[:, cs:512], ohc[:, hq, :], su[:, cs:512],
                                        start=(hq == 0 and first_sum[hq]),
                                        stop=(hq == HLOC - 1 and sp),
                                        skip_group_check=True)
                                    first_sum[hq] = False
                            del saved[jp]
                        nc.scalar.activation(ytn[:, h0, w * 512:(w + 1) * 512], ps_y0[:],
                                             mybir.ActivationFunctionType.Copy)
                        nc.scalar.activation(ytn[:, h1, w * 512:(w + 1) * 512], ps_y1[:],
                                             mybir.ActivationFunctionType.Copy)
                        # P3 gap filler: one og-stripe of the previous window
                        if w > 0:
                            for tcl in range(4):
                                emit_p3_tile(hp, 4 * (w - 1) + tcl)

                    # batched 1/sums for all 8 heads of this window
                    lgs = sm.tile([HLOC, 512], f32, tag="lgs")
                    nc.scalar.activation(lgs[:], ps_sums[:],
                                         mybir.ActivationFunctionType.Ln)
                    rec = sm.tile([HLOC, 512], bf16, tag="rec")
                    nc.scalar.activation(rec[:], lgs[:],
                                         mybir.ActivationFunctionType.Exp,
                                         scale=-1.0)
                    for hq in range(HLOC):
                        bcp = psc.tile([128, 512], f32, tag="sc")
                        nc.tensor.matmul(bcp[:], ohr[:, hq, :], rec[:])
                        nc.vector.tensor_mul(
                            ytn[:, hq, w * 512:(w + 1) * 512],
                            ytn[:, hq, w * 512:(w + 1) * 512], bcp[:])

                # final P3 stripe: window 3
                for og in range(4):
                    for tcl in range(4):
                        emit_p3_tile(og, 12 + tcl)

    nc.compile()
    return nc


_PROGRAM = None


def _get_program():
    global _PROGRAM
    if _PROGRAM is None:
        _PROGRAM = build_program()
    return _PROGRAM


def make_in_maps(x, W_qkv, W_out):
    in_maps = []
    for c in range(8):
        b, t = c // 2, c % 2
        xt = np.ascontiguousarray(x[b].T).astype(BF)
        wq = np.ascontiguousarray(W_qkv[:, t * 1024:(t + 1) * 1024]).astype(BF)
        wk = W_qkv[:, D + t * 256: D + (t + 1) * 256]
        wv = W_qkv[:, D + 512 + t * 256: D + 512 + (t + 1) * 256]
        wkv = np.ascontiguousarray(np.concatenate([wk, wv], axis=1)).astype(BF)
        wo = np.ascontiguousarray(W_out[t * 1024:(t + 1) * 1024, :]).astype(BF)
        in_maps.append({"xt": xt, "wq": wq, "wkv": wkv, "wo": wo})
    return in_maps


def kernel(x, W_qkv, W_out):
    from concourse.bass_utils import run_bass_kernel_spmd
    nc = _get_program()
    in_maps = make_in_maps(np.asarray(x, dtype=np.float32),
                           np.asarray(W_qkv, dtype=np.float32),
                           np.asarray(W_out, dtype=np.float32))
    res = run_bass_kernel_spmd(nc, in_maps, list(range(8)), trace=False)
    out = np.empty((B, S, D), dtype=np.float32)
    for b in range(B):
        out[b] = (res.results[2 * b]["out"].astype(np.float32)
                  + res.results[2 * b + 1]["out"].astype(np.float32))
    return out
